# revision 8
# baseline (speedup 1.0000x reference)
"""Additive (Bahdanau) attention context-vector kernel for Trainium2, 8-core SPMD.

Problem (per batch element b):
    encW = enc @ W_enc                      # (T_enc=512, D=256)
    decW = dec @ W_dec                      # (T_dec=256, D=256)
    alpha[s,t] = sum_d v[d] * tanh(encW[t,d] + decW[s,d])
    weights = softmax(alpha, axis=t)        # (256, 512)
    context = weights @ enc                 # (256, 256)

Sharding: data-parallel over batch B=8 -> one batch element per NeuronCore.

Per-core implementation:
  - encWT (D-part, T_enc-free) and decWT (D-part, T_dec-free) computed on PE
    (inputs transposed via PE transpose-mode with an identity tile).
  - Main loop: DVE tensor_scalar_add broadcasts decWT[:, s] over encWT into
    big (128, 4096) sum tiles (8 decoder steps per tile); ACT applies tanh in
    one big instruction; PE contracts with v via a shifted-window trick:
    Z = [0...0 | v_c | 0...0] so lhsT = Z[:, base-s : base+128-s] has v in
    column s only -> each matmul deposits alpha row s into PSUM partition s of
    a single accumulating bank (all other rows get +0).
  - Softmax per 128-row chunk straight off PSUM (ACT exp w/ -max bias).
  - context via PE after transposing weights back (PE transpose-mode).
"""

import numpy as np

_STATE = {}

TE = 512   # T_enc
TD = 256   # T_dec
DE = 256   # D_enc (attention dim)
DD = 512   # D_dec
P = 128
NCORES = 8


def _build():
    import concourse.bacc as bacc
    import concourse.tile as tile
    from concourse import mybir
    from concourse.masks import make_identity

    f32 = mybir.dt.float32
    f32r = mybir.dt.float32r
    AF = mybir.ActivationFunctionType
    Alu = mybir.AluOpType
    Ax = mybir.AxisListType

    nc = bacc.Bacc(None, target_bir_lowering=False)

    enc_d = nc.dram_tensor("enc", [TE, DE], f32, kind="ExternalInput").ap()
    dec_d = nc.dram_tensor("dec", [TD, DD], f32, kind="ExternalInput").ap()
    We_d = nc.dram_tensor("W_enc", [DE, DE], f32, kind="ExternalInput").ap()
    Wd_d = nc.dram_tensor("W_dec", [DD, DE], f32, kind="ExternalInput").ap()
    v_d = nc.dram_tensor("v", [DE, 1], f32, kind="ExternalInput").ap()
    ctx_d = nc.dram_tensor("context", [TD, DE], f32, kind="ExternalOutput").ap()
    w_d = nc.dram_tensor("weights", [TD, TE], f32, kind="ExternalOutput").ap()

    GRP = 8          # decoder steps fused per ACT instruction
    NG = P // GRP    # groups per 128-row chunk

    with tile.TileContext(nc) as tc:
        with tc.tile_pool(name="consts", bufs=1) as consts, \
             tc.tile_pool(name="small", bufs=4) as small, \
             tc.tile_pool(name="sums", bufs=3) as sums, \
             tc.tile_pool(name="tanhs", bufs=3) as tanhs, \
             tc.tile_pool(name="wexps", bufs=2) as wexps:

            ident = consts.tile([P, P], f32, name="ident", tag="ident")
            make_identity(nc, ident)

            encN = [consts.tile([P, DE], f32, name=f"encN{i}", tag=f"encN{i}")
                    for i in range(4)]
            encT = [consts.tile([P, TE], f32, name=f"encT{i}", tag=f"encT{i}")
                    for i in range(2)]
            encWT = [consts.tile([P, TE], f32, name=f"encWT{i}", tag=f"encWT{i}")
                     for i in range(2)]
            decWT = [consts.tile([P, TD], f32, name=f"decWT{i}", tag=f"decWT{i}")
                     for i in range(2)]
            Z = consts.tile([P, 2 * DE], f32r, name="Zsel", tag="Zsel")
            vstage = consts.tile([P, 2], f32, name="vstage", tag="vstage")
            weights_sb = [consts.tile([P, TE], f32, name=f"wsb{i}", tag=f"wsb{i}")
                          for i in range(2)]
            wT = [consts.tile([P, TD], f32, name=f"wT{i}", tag=f"wT{i}")
                  for i in range(4)]
            ctx_sb = [consts.tile([P, DE], f32, name=f"ctxsb{i}", tag=f"ctxsb{i}")
                      for i in range(2)]

            for i in range(4):
                nc.sync.dma_start(out=encN[i], in_=enc_d[i * P:(i + 1) * P, :])
            nc.sync.dma_start(out=vstage[:, 0:1], in_=v_d[0:P, :])
            nc.sync.dma_start(out=vstage[:, 1:2], in_=v_d[P:2 * P, :])

            # ---------- setup: transposes + projection matmuls ----------
            with tc.tile_pool(name="setup", bufs=1) as setup, \
                 tc.tile_pool(name="pst", bufs=2, space="PSUM") as pst, \
                 tc.tile_pool(name="psm", bufs=2, space="PSUM") as psm:

                decN = [setup.tile([P, DD], f32, name=f"decN{i}", tag=f"decN{i}")
                        for i in range(2)]
                decT = [setup.tile([P, TD], f32, name=f"decT{i}", tag=f"decT{i}")
                        for i in range(4)]
                WeN = [setup.tile([P, DE], f32, name=f"WeN{i}", tag=f"WeN{i}")
                       for i in range(2)]
                WdN = [setup.tile([P, DE], f32, name=f"WdN{i}", tag=f"WdN{i}")
                       for i in range(4)]

                zstage = setup.tile([P, 2 * DE], f32, name="zstage", tag="zstage")
                nc.vector.memset(zstage, 0.0)
                nc.vector.tensor_copy(Z, zstage)
                nc.vector.tensor_copy(Z[:, P:P + 1], vstage[:, 0:1])
                nc.vector.tensor_copy(Z[:, DE + P:DE + P + 1], vstage[:, 1:2])

                for i in range(2):
                    nc.sync.dma_start(out=decN[i], in_=dec_d[i * P:(i + 1) * P, :])
                for i in range(2):
                    nc.sync.dma_start(out=WeN[i], in_=We_d[i * P:(i + 1) * P, :])
                for i in range(4):
                    nc.sync.dma_start(out=WdN[i], in_=Wd_d[i * P:(i + 1) * P, :])

                # encT[ec][:, tcc*P:...] = encN[tcc][:, ec*P:...].T
                for tcc in range(4):
                    for ec in range(2):
                        pt = pst.tile([P, P], f32, name="pt", tag="pt")
                        nc.tensor.transpose(
                            pt, encN[tcc][:, ec * P:(ec + 1) * P], ident)
                        nc.vector.tensor_copy(
                            encT[ec][:, tcc * P:(tcc + 1) * P], pt)
                # decT[dc][:, sc*P:...] = decN[sc][:, dc*P:...].T
                for sc in range(2):
                    for dc in range(4):
                        pt = pst.tile([P, P], f32, name="pt", tag="pt")
                        nc.tensor.transpose(
                            pt, decN[sc][:, dc * P:(dc + 1) * P], ident)
                        nc.vector.tensor_copy(
                            decT[dc][:, sc * P:(sc + 1) * P], pt)

                # encWT[fc] = (W_enc.T @ enc.T)[fc chunk]
                for fc in range(2):
                    pm = psm.tile([P, TE], f32, name="pm_e", tag="pm_e")
                    for ec in range(2):
                        nc.tensor.matmul(
                            pm, lhsT=WeN[ec][:, fc * P:(fc + 1) * P],
                            rhs=encT[ec], start=(ec == 0), stop=(ec == 1))
                    nc.vector.tensor_copy(encWT[fc], pm)
                # decWT[fc] = (W_dec.T @ dec.T)[fc chunk]
                for fc in range(2):
                    pm = psm.tile([P, TD], f32, name="pm_d", tag="pm_d")
                    for dc in range(4):
                        nc.tensor.matmul(
                            pm, lhsT=WdN[dc][:, fc * P:(fc + 1) * P],
                            rhs=decT[dc], start=(dc == 0), stop=(dc == 3))
                    nc.vector.tensor_copy(decWT[fc], pm)

            # ---------- main loop: tanh + v-contraction + softmax ----------
            with tc.tile_pool(name="psa", bufs=2, space="PSUM") as psa:
                for sc in range(2):
                    alpha_ps = psa.tile([P, TE], f32, name="alpha", tag="alpha")
                    for g in range(NG):
                        for c in range(2):
                            sb = sums.tile([P, GRP * TE], f32, name="sb", tag="sb")
                            for k in range(GRP):
                                si = g * GRP + k
                                col = sc * P + si
                                nc.vector.tensor_scalar_add(
                                    out=sb[:, k * TE:(k + 1) * TE],
                                    in0=encWT[c],
                                    scalar1=decWT[c][:, col:col + 1])
                            th = tanhs.tile([P, GRP * TE], f32r, name="th", tag="th")
                            nc.scalar.activation(th, sb, AF.Tanh)
                            for k in range(GRP):
                                si = g * GRP + k
                                base = c * DE + P - si
                                nc.tensor.matmul(
                                    alpha_ps,
                                    lhsT=Z[:, base:base + P],
                                    rhs=th[:, k * TE:(k + 1) * TE],
                                    start=(g == 0 and c == 0 and k == 0),
                                    stop=(g == NG - 1 and c == 1 and k == GRP - 1),
                                    skip_group_check=True)
                    # softmax over t (free axis)
                    rmax = small.tile([P, 1], f32, name="rmax", tag="rmax")
                    nc.vector.tensor_reduce(
                        rmax, alpha_ps, axis=Ax.X, op=Alu.max, negate=True)
                    wexp = wexps.tile([P, TE], f32, name="wexp", tag="wexp")
                    nc.scalar.activation(wexp, alpha_ps, AF.Exp, bias=rmax)
                    rsum = small.tile([P, 1], f32, name="rsum", tag="rsum")
                    nc.vector.tensor_reduce(rsum, wexp, axis=Ax.X, op=Alu.add)
                    rinv = small.tile([P, 1], f32, name="rinv", tag="rinv")
                    nc.vector.reciprocal(rinv, rsum)
                    nc.vector.tensor_scalar_mul(
                        out=weights_sb[sc], in0=wexp, scalar1=rinv)
                    nc.sync.dma_start(
                        out=w_d[sc * P:(sc + 1) * P, :], in_=weights_sb[sc])

            # ---------- context = weights @ enc ----------
            with tc.tile_pool(name="psw", bufs=2, space="PSUM") as psw, \
                 tc.tile_pool(name="psc", bufs=2, space="PSUM") as psc:
                for sc in range(2):
                    for tcc in range(4):
                        pw = psw.tile([P, P], f32, name="pw", tag="pw")
                        nc.tensor.transpose(
                            pw, weights_sb[sc][:, tcc * P:(tcc + 1) * P], ident)
                        nc.vector.tensor_copy(
                            wT[tcc][:, sc * P:(sc + 1) * P], pw)
                for sc in range(2):
                    pc = psc.tile([P, DE], f32, name="pc", tag="pc")
                    for tcc in range(4):
                        nc.tensor.matmul(
                            pc, lhsT=wT[tcc][:, sc * P:(sc + 1) * P],
                            rhs=encN[tcc], start=(tcc == 0), stop=(tcc == 3))
                    nc.vector.tensor_copy(ctx_sb[sc], pc)
                    nc.sync.dma_start(
                        out=ctx_d[sc * P:(sc + 1) * P, :], in_=ctx_sb[sc])

    nc.finalize()
    return nc


def _get_nc():
    if "nc" not in _STATE:
        _STATE["nc"] = _build()
    return _STATE["nc"]


def kernel(encoder_outputs, decoder_states, W_enc, W_dec, v):
    from concourse.bass_utils import run_bass_kernel_spmd

    enc = np.ascontiguousarray(np.asarray(encoder_outputs, dtype=np.float32))
    dec = np.ascontiguousarray(np.asarray(decoder_states, dtype=np.float32))
    We = np.ascontiguousarray(np.asarray(W_enc, dtype=np.float32))
    Wd = np.ascontiguousarray(np.asarray(W_dec, dtype=np.float32))
    vv = np.ascontiguousarray(np.asarray(v, dtype=np.float32))

    nc = _get_nc()
    in_maps = [
        {"enc": enc[i], "dec": dec[i], "W_enc": We, "W_dec": Wd, "v": vv}
        for i in range(NCORES)
    ]
    res = run_bass_kernel_spmd(nc, in_maps, list(range(NCORES)))
    context = np.stack([res.results[i]["context"] for i in range(NCORES)])
    weights = np.stack([res.results[i]["weights"] for i in range(NCORES)])
    return (context, weights)
